# revision 27
# baseline (speedup 1.0000x reference)
"""Multi-head causal attention block (B=2, S=2048, F=1024, H=16, D=64)
on 8 TRN2 NeuronCores.

Sharding: core = 4*b + g  (b = batch 0..1, g = head-group 0..3, 4 heads each).
Each core computes, for its batch and its 4 heads:
  qkv projection (columns of w_attn for its heads), causal attention,
  and the partial output projection (rows of w_proj for its heads).
Host sums the 4 per-group partials per batch and adds the bias constant
(b_proj + b_attn_v @ w_proj, which is token-independent).

On-chip dataflow ("orientation B" — scores transposed, no P-transposes of
the attention weights):
  xT   [f, s]   via PE transposes (4 per PSUM bank, batched copy-out)
  qkT  [dim, s] = wqk^T @ xT; chunks [q_h0|q_h1],[q_h2|q_h3],[k_h0|k_h1],[k_h2|k_h3]
  v    [s, d]   direct orientation, +ones column per head (denominator row)
  attention per head h, sq-chunk c (512 wide), sk tile t<=diag:
    sT = matmul(lhsT=kT_h[:,t], rhs=qT_h[:,chunk])  [sk=128, sq<=512] PSUM
    (pairs of t share one 2-bank PSUM tile; one exp op per pair)
    exp on ACT -> SBUF f32r; causal triangle zeroed by GPSIMD affine_select
    zT'[65, 512] += v_ones_h[:,t].T @ expP  (PSUM accumulate; row 64 = denom)
    normalize: z = zT'[:64] * bcast(approx_recip(zT'[64]))
  out partial [s, f] = zTm.T @ wp

Diag tiles compute only the valid sq range (width 512-off), so there is no
wasted score/exp/AV work beyond the masked 128x128 triangle.

Everything is emitted chunk-pipelined (transpose(c) -> proj(c) ->
attention(c) -> outproj(c)) so the Tile scheduler can overlap phases and
keep the PE dense (HAM stays un-throttled).

All matmuls run in float32r (full-rate fp32; ~2^-14 operand rounding).
"""

import numpy as np

import concourse.mybir as mybir
import concourse.tile as tile
from concourse import bacc
from concourse.bass_utils import run_bass_kernel_spmd
from concourse.masks import make_identity

B, S, F, H, D = 2, 2048, 1024, 16, 64
P = 128
NCORES = 8
HPC = 4  # heads per core
GD = HPC * D  # 256 dims per head group
ST = S // P  # 16 sequence tiles
FC = F // P  # 8 feature chunks
SQC = 4  # sq chunks of 512
CW = 512  # chunk width
NEG = -1.0e9

f32 = mybir.dt.float32
f32r = mybir.dt.float32r

_cached_nc = None


def build_nc():
    nc = bacc.Bacc("TRN2", target_bir_lowering=False, debug=False,
                   num_devices=NCORES)
    x = nc.dram_tensor("x", [S, F], f32, kind="ExternalInput")
    wqk = nc.dram_tensor("wqk", [F, 2 * GD], f32, kind="ExternalInput")
    wv = nc.dram_tensor("wv", [F, GD], f32, kind="ExternalInput")
    wp = nc.dram_tensor("wp", [GD, F], f32, kind="ExternalInput")
    bqk = nc.dram_tensor("bqk", [P, 4], f32, kind="ExternalInput")
    out = nc.dram_tensor("out", [S, F], f32, kind="ExternalOutput")

    with tile.TileContext(nc) as tc:
        with (
            tc.tile_pool(name="consts", bufs=1) as consts,
            tc.tile_pool(name="stage", bufs=1) as stage,
            tc.tile_pool(name="work", bufs=2) as work,
            tc.tile_pool(name="eps", bufs=4) as eps,
            tc.tile_pool(name="norm", bufs=1) as norm,
            tc.tile_pool(name="ps_s", bufs=2, space="PSUM") as ps_s,
            tc.tile_pool(name="ps_z", bufs=2, space="PSUM") as ps_z,
            tc.tile_pool(name="ps_m", bufs=2, space="PSUM") as ps_m,
        ):
            # ---- constants ----
            ident = consts.tile([P, P], f32)
            make_identity(nc, ident[:])
            ones = consts.tile([P, 1], f32)
            nc.vector.memset(ones[:], 1.0)
            bqk_sb = consts.tile([P, 4], f32)
            nc.sync.dma_start(bqk_sb[:], bqk[:])
            # additive causal triangle: keep iff jloc >= i, else -1e9
            mask128 = consts.tile([P, P], f32)
            nc.gpsimd.memset(mask128[:], 0.0)
            nc.gpsimd.affine_select(
                out=mask128[:], in_=mask128[:],
                compare_op=mybir.AluOpType.is_ge,
                fill=NEG, base=0,
                pattern=[[1, P]], channel_multiplier=-1,
            )

            # ---- round weights to f32r (streamed through small tiles) ----
            wqk_r = stage.tile([P, FC, 2 * GD], f32r, tag="wqk_r", name="wqk_r")
            wv_r = stage.tile([P, FC, GD], f32r, tag="wv_r", name="wv_r")
            wp_r = stage.tile([P, 2, F], f32r, tag="wp_r", name="wp_r")
            def load_wqk_task(fc):
                wt = work.tile([P, 2 * GD], f32, tag="wtmp", name="wt_qk")
                nc.sync.dma_start(wt[:], wqk[fc * P:(fc + 1) * P, :])
                nc.vector.tensor_copy(wqk_r[:, fc, :], wt[:])

            def load_wv_task(fc):
                wt = work.tile([P, 2 * GD], f32, tag="wtmp", name="wt_v")
                nc.sync.dma_start(wt[:, :GD], wv[fc * P:(fc + 1) * P, :])
                nc.vector.tensor_copy(wv_r[:, fc, :], wt[:, :GD])

            def load_wp_task(cc):
                for hh in range(2):
                    wt = work.tile([P, 2 * GD], f32, tag="wtmp", name="wt_p")
                    nc.sync.dma_start(
                        wt[:], wp[cc * P:(cc + 1) * P,
                                  hh * CW:(hh + 1) * CW])
                    nc.vector.tensor_copy(
                        wp_r[:, cc, hh * CW:(hh + 1) * CW], wt[:])

            # ---- persistent activations ----
            xT = stage.tile([P, FC, S], f32r, tag="xT", name="xT")
            qkT = stage.tile([P, 4, S], f32r, tag="qkT", name="qkT")
            vt = stage.tile([P, HPC, ST, D + 1], f32r, tag="vt", name="vt")
            zTm = stage.tile([P, 2, S], f32r, tag="zTm", name="zTm")
            for h in range(HPC):
                nc.vector.tensor_copy(
                    vt[:, h, :, D:D + 1],
                    ones[:, None, :].to_broadcast((P, ST, 1)),
                )

            def transpose_task(c, tt):
                t = 4 * c + tt
                xt_ = work.tile([P, F], f32, tag="xtile", name="xtile")
                hw = F // 2
                for half in range(2):
                    nc.sync.dma_start(
                        xt_[:, half * hw:(half + 1) * hw],
                        x[t * P:(t + 1) * P, half * hw:(half + 1) * hw],
                    )
                for half in range(2):
                    pp = ps_m.tile([P, CW], f32, tag="mps", name="tps")
                    for q in range(4):
                        fc = half * 4 + q
                        nc.tensor.transpose(
                            pp[:, q * P:(q + 1) * P],
                            xt_[:, fc * P:(fc + 1) * P],
                            ident[:],
                        )
                    nc.vector.tensor_copy(
                        xT[:, half * 4:half * 4 + 4,
                           t * P:(t + 1) * P],
                        pp[:].rearrange("p (f q) -> p f q", f=4),
                    )

            def qkproj_task(c, oc):
                    pp = ps_m.tile([P, CW], f32, tag="mps", name="qkps")
                    for fc in range(FC):
                        nc.tensor.matmul(
                            pp[:],
                            wqk_r[:, fc, oc * P:(oc + 1) * P],
                            xT[:, fc, c * CW:(c + 1) * CW],
                            start=(fc == 0), stop=(fc == FC - 1),
                        )
                    nc.vector.tensor_tensor(
                        qkT[:, oc, c * CW:(c + 1) * CW], pp[:],
                        bqk_sb[:, oc:oc + 1].to_broadcast((P, CW)),
                        mybir.AluOpType.add,
                    )

            def vproj_task(c, tt):
                    t = 4 * c + tt
                    pp = ps_m.tile([P, GD], f32, tag="mps", name="vps")
                    for fc in range(FC):
                        nc.tensor.matmul(
                            pp[:],
                            xT[:, fc, t * P:(t + 1) * P],
                            wv_r[:, fc, :],
                            start=(fc == 0), stop=(fc == FC - 1),
                        )
                    nc.vector.tensor_copy(
                        vt[:, :, t, :D],
                        pp[:].rearrange("p (h d) -> p h d", h=HPC),
                    )

            def av(zp, h, t, ep_ap, col0, ncols, start, stop):
                nc.tensor.matmul(
                    zp[:D + 1, col0:col0 + ncols],
                    vt[:, h, t, :],
                    ep_ap,
                    start=start, stop=stop,
                    skip_group_check=True,
                )

            def scores(sp_ap, h, t, c, q0, qw):
                lo = (h % 2) * D
                nc.tensor.matmul(
                    sp_ap,
                    qkT[lo:lo + D, 2 + h // 2, t * P:(t + 1) * P],
                    qkT[lo:lo + D, h // 2, c * CW + q0:c * CW + q0 + qw],
                    start=True, stop=True,
                    skip_group_check=True,
                )

            def diag_mask(sp_ap):
                nc.vector.tensor_add(sp_ap, sp_ap, mask128[:])

            def attention(c, fillers):
                # insertion points: one after each head's exp emission
                npts = 2 * (2 * c + 2) * 2
                state = {"fi": 0, "pt": 0}

                def fill():
                    state["pt"] += 1
                    left = npts - state["pt"] + 1
                    remaining = len(fillers) - state["fi"]
                    k = (remaining + left - 1) // left if left > 0 else remaining
                    for _ in range(k):
                        fillers[state["fi"]]()
                        state["fi"] += 1

                for hp in range(2):
                    heads = (2 * hp, 2 * hp + 1)
                    zps = [
                        ps_z.tile([P, CW], f32, tag="zps", name=f"zps{i}")
                        for i in range(2)
                    ]
                    # off-diagonal pairs (full width); both heads' score
                    # matmuls issued adjacently so the K=64 matmuls pack
                    # into disjoint PE row groups and run concurrently.
                    for pair in range(2 * c):
                        t0, t1 = 2 * pair, 2 * pair + 1
                        sp2 = [
                            ps_s.tile([P, 2 * CW], f32, tag="sps",
                                      name=f"sps{i}")
                            for i in range(2)
                        ]
                        for i, h in enumerate(heads):
                            scores(sp2[i][:, 0:CW], h, t0, c, 0, CW)
                            scores(sp2[i][:, CW:2 * CW], h, t1, c, 0, CW)
                        ep2 = []
                        for i, h in enumerate(heads):
                            ep = eps.tile([P, 2 * CW], f32r, tag="ep",
                                          name=f"ep{i}")
                            nc.scalar.activation(
                                ep[:], sp2[i][:],
                                mybir.ActivationFunctionType.Exp,
                            )
                            ep2.append(ep)
                        fill()
                        first = (t0 == 0)
                        for i, h in enumerate(heads):
                            av(zps[i], h, t0, ep2[i][:, 0:CW], 0, CW,
                               first, False)
                            av(zps[i], h, t1, ep2[i][:, CW:2 * CW], 0, CW,
                               False, False)
                        fill()
                    # diagonal pairs: widths (512, 384) and (256, 128)
                    for dp in range(2):
                        ta, tb = 4 * c + 2 * dp, 4 * c + 2 * dp + 1
                        offa, offb = 2 * dp * P, (2 * dp + 1) * P
                        wa, wb = CW - offa, CW - offb
                        sp2 = [
                            ps_s.tile([P, 2 * CW], f32, tag="sps",
                                      name=f"sps{i}")
                            for i in range(2)
                        ]
                        for i, h in enumerate(heads):
                            scores(sp2[i][:, 0:wa], h, ta, c, offa, wa)
                            scores(sp2[i][:, wa:wa + wb], h, tb, c, offb, wb)
                        ep2 = []
                        for i, h in enumerate(heads):
                            diag_mask(sp2[i][:, 0:P])
                            diag_mask(sp2[i][:, wa:wa + P])
                            ep = eps.tile([P, 2 * CW], f32r, tag="ep",
                                          name=f"ep{i}")
                            nc.scalar.activation(
                                ep[:, 0:wa + wb], sp2[i][:, 0:wa + wb],
                                mybir.ActivationFunctionType.Exp,
                            )
                            ep2.append(ep)
                        fill()
                        first = (c == 0 and dp == 0)
                        for i, h in enumerate(heads):
                            av(zps[i], h, ta, ep2[i][:, 0:wa], offa, wa,
                               first, False)
                            av(zps[i], h, tb, ep2[i][:, wa:wa + wb], offb,
                               wb, False, (dp == 1))
                        fill()
                    # normalize
                    for i, h in enumerate(heads):
                        den = norm.tile([1, CW], f32, tag="den", name="den")
                        nc.vector.tensor_copy(den[:], zps[i][D:D + 1, :])
                        rec = norm.tile([1, CW], f32, tag="rec", name="rec")
                        nc.vector.reciprocal_approx_fast(rec[:], den[:])
                        recb = norm.tile([D, CW], f32, tag="recb",
                                         name="recb")
                        nc.gpsimd.partition_broadcast(recb[:], rec[:])
                        lo = (h % 2) * D
                        nc.vector.tensor_mul(
                            zTm[lo:lo + D, h // 2, c * CW:(c + 1) * CW],
                            zps[i][:D], recb[:],
                        )
                while state["fi"] < len(fillers):
                    fillers[state["fi"]]()
                    state["fi"] += 1

            def outproj_task(c, tt, n):
                    t = 4 * c + tt
                    osb = work.tile([P, CW], f32, tag="osb", name="osb")
                    pp = ps_m.tile([P, CW], f32, tag="mps", name="ops")
                    for cc in range(2):
                        nc.tensor.matmul(
                            pp[:],
                            zTm[:, cc, t * P:(t + 1) * P],
                            wp_r[:, cc, n * CW:(n + 1) * CW],
                            start=(cc == 0), stop=(cc == 1),
                        )
                    nc.vector.tensor_copy(osb[:], pp[:])
                    nc.sync.dma_start(
                        out[t * P:(t + 1) * P, n * CW:(n + 1) * CW], osb[:]
                    )

            def prep_tasks(c):
                tasks = [
                    (lambda tt=tt: transpose_task(c, tt)) for tt in range(4)
                ]
                tasks += [(lambda oc=oc: qkproj_task(c, oc)) for oc in range(4)]
                tasks += [(lambda tt=tt: vproj_task(c, tt)) for tt in range(4)]
                return tasks

            # chunk 0 prep up front, then software-pipeline: during
            # attention(c), weave in outproj(c-1) and all prep for c+1.
            p0 = prep_tasks(0)
            wq_tasks = [(lambda fc=fc: load_wqk_task(fc)) for fc in range(FC)]
            wv_tasks = [(lambda fc=fc: load_wv_task(fc)) for fc in range(FC)]
            # transposes(0) first, weights woven behind them, then projs
            for task in p0[:4]:
                task()
            for task in wq_tasks + wv_tasks:
                task()
            for task in p0[4:]:
                task()
            # outproj(c) is shifted as late as possible so the long final
            # attention chunks (most insertion points) have filler work:
            # att0: wp+prep1, att1: prep2, att2: prep3+out0, att3: out1+out2
            out_t = [
                [(lambda tt=tt, cp=cp, n=n: outproj_task(cp, tt, n))
                 for tt in range(4) for n in range(2)]
                for cp in range(SQC)
            ]
            for c in range(SQC):
                fillers = []
                if c == 0:
                    fillers += [(lambda cc=cc: load_wp_task(cc))
                                for cc in range(2)]
                if c + 1 < SQC:
                    fillers += prep_tasks(c + 1)
                if c == 3:
                    fillers += out_t[0] + out_t[1] + out_t[2]
                attention(c, fillers)
            for task in out_t[3]:
                task()
    nc.compile()
    return nc


def make_in_maps(x, w_attn, b_attn, w_proj):
    x = np.ascontiguousarray(np.asarray(x, dtype=np.float32))
    w_attn = np.asarray(w_attn, dtype=np.float32)
    b_attn = np.asarray(b_attn, dtype=np.float32)
    w_proj = np.ascontiguousarray(np.asarray(w_proj, dtype=np.float32))
    scale = np.float32(1.0 / np.sqrt(D))
    in_maps = []
    for core in range(NCORES):
        b, g = divmod(core, 4)
        sl = slice(g * GD, (g + 1) * GD)
        wq = w_attn[:, sl] * scale
        wk = w_attn[:, F + g * GD:F + (g + 1) * GD]
        wqkm = np.ascontiguousarray(
            np.concatenate([wq, wk], axis=1), dtype=np.float32
        )
        wvm = np.ascontiguousarray(
            w_attn[:, 2 * F + g * GD:2 * F + (g + 1) * GD]
        )
        wpg = np.ascontiguousarray(w_proj[sl, :])
        bq = b_attn[sl] * scale
        bk = b_attn[F + g * GD:F + (g + 1) * GD]
        bqkm = np.ascontiguousarray(
            np.concatenate([bq, bk]).reshape(4, P).T, dtype=np.float32
        )
        in_maps.append(
            {"x": np.ascontiguousarray(x[b]), "wqk": wqkm, "wv": wvm,
             "wp": wpg, "bqk": bqkm}
        )
    return in_maps


def assemble(results, b_attn, b_proj, w_proj):
    b_attn = np.asarray(b_attn, dtype=np.float64)
    b_proj = np.asarray(b_proj, dtype=np.float64)
    w_proj = np.asarray(w_proj, dtype=np.float64)
    const = b_attn[2 * F:] @ w_proj + b_proj  # token-independent v-bias term
    full = np.empty((B, S, F), dtype=np.float32)
    for b in range(B):
        acc = results[4 * b]["out"].astype(np.float64)
        for g in range(1, 4):
            acc = acc + results[4 * b + g]["out"]
        full[b] = (acc + const).astype(np.float32)
    return full


def kernel(x, w_attn, b_attn, w_proj, b_proj):
    global _cached_nc
    if _cached_nc is None:
        _cached_nc = build_nc()
    in_maps = make_in_maps(x, w_attn, b_attn, w_proj)
    res = run_bass_kernel_spmd(
        _cached_nc, in_maps, core_ids=list(range(NCORES))
    )
    return assemble(res.results, b_attn, b_proj, w_proj)


# revision 28
# speedup vs baseline: 1.0718x; 1.0718x over previous
"""Multi-head causal attention block (B=2, S=2048, F=1024, H=16, D=64)
on 8 TRN2 NeuronCores.

Sharding: core = 4*b + g  (b = batch 0..1, g = head-group 0..3, 4 heads each).
Each core computes, for its batch and its 4 heads:
  qkv projection (columns of w_attn for its heads), causal attention,
  and the partial output projection (rows of w_proj for its heads).
Host sums the 4 per-group partials per batch and adds the bias constant
(b_proj + b_attn_v @ w_proj, which is token-independent).

On-chip dataflow ("orientation B" — scores transposed, no P-transposes of
the attention weights):
  xT   [f, s]   via PE transposes (4 per PSUM bank, batched copy-out)
  qkT  [dim, s] = wqk^T @ xT; chunks [q_h0|q_h1],[q_h2|q_h3],[k_h0|k_h1],[k_h2|k_h3]
  v    [s, d]   direct orientation, +ones column per head (denominator row)
  attention per head h, sq-chunk c (512 wide), sk tile t<=diag:
    sT = matmul(lhsT=kT_h[:,t], rhs=qT_h[:,chunk])  [sk=128, sq<=512] PSUM
    (pairs of t share one 2-bank PSUM tile; one exp op per pair)
    exp on ACT -> SBUF f32r; causal triangle zeroed by GPSIMD affine_select
    zT'[65, 512] += v_ones_h[:,t].T @ expP  (PSUM accumulate; row 64 = denom)
    normalize: z = zT'[:64] * bcast(approx_recip(zT'[64]))
  out partial [s, f] = zTm.T @ wp

Diag tiles compute only the valid sq range (width 512-off), so there is no
wasted score/exp/AV work beyond the masked 128x128 triangle.

Everything is emitted chunk-pipelined (transpose(c) -> proj(c) ->
attention(c) -> outproj(c)) so the Tile scheduler can overlap phases and
keep the PE dense (HAM stays un-throttled).

All matmuls run in float32r (full-rate fp32; ~2^-14 operand rounding).
"""

import numpy as np

import concourse.mybir as mybir
import concourse.tile as tile
from concourse import bacc
from concourse.bass_utils import run_bass_kernel_spmd
from concourse.masks import make_identity

B, S, F, H, D = 2, 2048, 1024, 16, 64
P = 128
NCORES = 8
HPC = 4  # heads per core
GD = HPC * D  # 256 dims per head group
ST = S // P  # 16 sequence tiles
FC = F // P  # 8 feature chunks
SQC = 4  # sq chunks of 512
CW = 512  # chunk width
NEG = -1.0e9

f32 = mybir.dt.float32
f32r = mybir.dt.float32r

_cached_nc = None


def build_nc():
    nc = bacc.Bacc("TRN2", target_bir_lowering=False, debug=False,
                   num_devices=NCORES)
    x = nc.dram_tensor("x", [S, F], f32, kind="ExternalInput")
    wqk = nc.dram_tensor("wqk", [F, 2 * GD], f32, kind="ExternalInput")
    wv = nc.dram_tensor("wv", [F, GD], f32, kind="ExternalInput")
    wp = nc.dram_tensor("wp", [GD, F], f32, kind="ExternalInput")
    bqk = nc.dram_tensor("bqk", [P, 4], f32, kind="ExternalInput")
    out = nc.dram_tensor("out", [S, F], f32, kind="ExternalOutput")

    with tile.TileContext(nc) as tc:
        with (
            tc.tile_pool(name="consts", bufs=1) as consts,
            tc.tile_pool(name="stage", bufs=1) as stage,
            tc.tile_pool(name="work", bufs=2) as work,
            tc.tile_pool(name="eps", bufs=4) as eps,
            tc.tile_pool(name="norm", bufs=1) as norm,
            tc.tile_pool(name="ps_s", bufs=2, space="PSUM") as ps_s,
            tc.tile_pool(name="ps_z", bufs=2, space="PSUM") as ps_z,
            tc.tile_pool(name="ps_m", bufs=2, space="PSUM") as ps_m,
        ):
            # ---- constants ----
            ident = consts.tile([P, P], f32)
            make_identity(nc, ident[:])
            ones = consts.tile([P, 1], f32)
            nc.vector.memset(ones[:], 1.0)
            bqk_sb = consts.tile([P, 4], f32)
            nc.sync.dma_start(bqk_sb[:], bqk[:])
            # additive causal triangle: keep iff jloc >= i, else -1e9
            mask128 = consts.tile([P, P], f32)
            nc.gpsimd.memset(mask128[:], 0.0)
            nc.gpsimd.affine_select(
                out=mask128[:], in_=mask128[:],
                compare_op=mybir.AluOpType.is_ge,
                fill=NEG, base=0,
                pattern=[[1, P]], channel_multiplier=-1,
            )

            # ---- round weights to f32r (streamed through small tiles) ----
            wqk_r = stage.tile([P, FC, 2 * GD], f32r, tag="wqk_r", name="wqk_r")
            wv_r = stage.tile([P, FC, GD], f32r, tag="wv_r", name="wv_r")
            wp_r = stage.tile([P, 2, F], f32r, tag="wp_r", name="wp_r")
            def load_wqk_task(fc):
                wt = work.tile([P, 2 * GD], f32, tag="wtmp", name="wt_qk")
                nc.sync.dma_start(wt[:], wqk[fc * P:(fc + 1) * P, :])
                nc.vector.tensor_copy(wqk_r[:, fc, :], wt[:])

            def load_wv_task(fc):
                wt = work.tile([P, 2 * GD], f32, tag="wtmp", name="wt_v")
                nc.sync.dma_start(wt[:, :GD], wv[fc * P:(fc + 1) * P, :])
                nc.vector.tensor_copy(wv_r[:, fc, :], wt[:, :GD])

            def load_wp_task(cc):
                for hh in range(2):
                    wt = work.tile([P, 2 * GD], f32, tag="wtmp", name="wt_p")
                    nc.sync.dma_start(
                        wt[:], wp[cc * P:(cc + 1) * P,
                                  hh * CW:(hh + 1) * CW])
                    nc.vector.tensor_copy(
                        wp_r[:, cc, hh * CW:(hh + 1) * CW], wt[:])

            # ---- persistent activations ----
            xT = stage.tile([P, FC, S], f32r, tag="xT", name="xT")
            qkT = stage.tile([P, 4, S], f32r, tag="qkT", name="qkT")
            vt = stage.tile([P, HPC, ST, D + 1], f32r, tag="vt", name="vt")
            zTm = stage.tile([P, 2, S], f32r, tag="zTm", name="zTm")
            for h in range(HPC):
                nc.vector.tensor_copy(
                    vt[:, h, :, D:D + 1],
                    ones[:, None, :].to_broadcast((P, ST, 1)),
                )

            def transpose_task(c, tt):
                t = 4 * c + tt
                xt_ = work.tile([P, F], f32, tag="xtile", name="xtile")
                hw = F // 2
                for half in range(2):
                    nc.sync.dma_start(
                        xt_[:, half * hw:(half + 1) * hw],
                        x[t * P:(t + 1) * P, half * hw:(half + 1) * hw],
                    )
                for half in range(2):
                    pp = ps_m.tile([P, CW], f32, tag="mps", name="tps")
                    for q in range(4):
                        fc = half * 4 + q
                        nc.tensor.transpose(
                            pp[:, q * P:(q + 1) * P],
                            xt_[:, fc * P:(fc + 1) * P],
                            ident[:],
                        )
                    nc.vector.tensor_copy(
                        xT[:, half * 4:half * 4 + 4,
                           t * P:(t + 1) * P],
                        pp[:].rearrange("p (f q) -> p f q", f=4),
                    )

            def qkproj_task(c, oc):
                    pp = ps_m.tile([P, CW], f32, tag="mps", name="qkps")
                    for fc in range(FC):
                        nc.tensor.matmul(
                            pp[:],
                            wqk_r[:, fc, oc * P:(oc + 1) * P],
                            xT[:, fc, c * CW:(c + 1) * CW],
                            start=(fc == 0), stop=(fc == FC - 1),
                        )
                    nc.vector.tensor_tensor(
                        qkT[:, oc, c * CW:(c + 1) * CW], pp[:],
                        bqk_sb[:, oc:oc + 1].to_broadcast((P, CW)),
                        mybir.AluOpType.add,
                    )

            def vproj_task(c, tt):
                    t = 4 * c + tt
                    pp = ps_m.tile([P, GD], f32, tag="mps", name="vps")
                    for fc in range(FC):
                        nc.tensor.matmul(
                            pp[:],
                            xT[:, fc, t * P:(t + 1) * P],
                            wv_r[:, fc, :],
                            start=(fc == 0), stop=(fc == FC - 1),
                        )
                    nc.vector.tensor_copy(
                        vt[:, :, t, :D],
                        pp[:].rearrange("p (h d) -> p h d", h=HPC),
                    )

            def av(zp, h, t, ep_ap, col0, ncols, start, stop):
                nc.tensor.matmul(
                    zp[:D + 1, col0:col0 + ncols],
                    vt[:, h, t, :],
                    ep_ap,
                    start=start, stop=stop,
                    skip_group_check=True,
                )

            def scores(sp_ap, h, t, c, q0, qw):
                lo = (h % 2) * D
                nc.tensor.matmul(
                    sp_ap,
                    qkT[lo:lo + D, 2 + h // 2, t * P:(t + 1) * P],
                    qkT[lo:lo + D, h // 2, c * CW + q0:c * CW + q0 + qw],
                    start=True, stop=True,
                    skip_group_check=True,
                )

            def diag_mask(sp_ap):
                nc.vector.tensor_add(sp_ap, sp_ap, mask128[:])

            def attention(c, fillers):
                # insertion points: one after each head's exp emission
                npts = 2 * (2 * c + 2) * 2
                state = {"fi": 0, "pt": 0}

                def fill():
                    state["pt"] += 1
                    left = npts - state["pt"] + 1
                    remaining = len(fillers) - state["fi"]
                    k = (remaining + left - 1) // left if left > 0 else remaining
                    for _ in range(k):
                        fillers[state["fi"]]()
                        state["fi"] += 1

                for hp in range(2):
                    heads = (2 * hp, 2 * hp + 1)
                    zps = [
                        ps_z.tile([P, CW], f32, tag="zps", name=f"zps{i}")
                        for i in range(2)
                    ]
                    # off-diagonal pairs (full width); both heads' score
                    # matmuls issued adjacently so the K=64 matmuls pack
                    # into disjoint PE row groups and run concurrently.
                    for pair in range(2 * c):
                        t0, t1 = 2 * pair, 2 * pair + 1
                        sp2 = [
                            ps_s.tile([P, 2 * CW], f32, tag="sps",
                                      name=f"sps{i}")
                            for i in range(2)
                        ]
                        for i, h in enumerate(heads):
                            scores(sp2[i][:, 0:CW], h, t0, c, 0, CW)
                            scores(sp2[i][:, CW:2 * CW], h, t1, c, 0, CW)
                        ep2 = []
                        for i, h in enumerate(heads):
                            ep = eps.tile([P, 2 * CW], f32r, tag="ep",
                                          name=f"ep{i}")
                            nc.scalar.activation(
                                ep[:], sp2[i][:],
                                mybir.ActivationFunctionType.Exp,
                            )
                            ep2.append(ep)
                        fill()
                        first = (t0 == 0)
                        for i, h in enumerate(heads):
                            av(zps[i], h, t0, ep2[i][:, 0:CW], 0, CW,
                               first, False)
                            av(zps[i], h, t1, ep2[i][:, CW:2 * CW], 0, CW,
                               False, False)
                        fill()
                    # diagonal pairs: widths (512, 384) and (256, 128)
                    for dp in range(2):
                        ta, tb = 4 * c + 2 * dp, 4 * c + 2 * dp + 1
                        offa, offb = 2 * dp * P, (2 * dp + 1) * P
                        wa, wb = CW - offa, CW - offb
                        sp2 = [
                            ps_s.tile([P, 2 * CW], f32, tag="sps",
                                      name=f"sps{i}")
                            for i in range(2)
                        ]
                        for i, h in enumerate(heads):
                            scores(sp2[i][:, 0:wa], h, ta, c, offa, wa)
                            scores(sp2[i][:, wa:wa + wb], h, tb, c, offb, wb)
                        ep2 = []
                        for i, h in enumerate(heads):
                            diag_mask(sp2[i][:, 0:P])
                            diag_mask(sp2[i][:, wa:wa + P])
                            ep = eps.tile([P, 2 * CW], f32r, tag="ep",
                                          name=f"ep{i}")
                            nc.scalar.activation(
                                ep[:, 0:wa + wb], sp2[i][:, 0:wa + wb],
                                mybir.ActivationFunctionType.Exp,
                            )
                            ep2.append(ep)
                        fill()
                        first = (c == 0 and dp == 0)
                        for i, h in enumerate(heads):
                            av(zps[i], h, ta, ep2[i][:, 0:wa], offa, wa,
                               first, False)
                            av(zps[i], h, tb, ep2[i][:, wa:wa + wb], offb,
                               wb, False, (dp == 1))
                        fill()
                    # normalize
                    for i, h in enumerate(heads):
                        den = norm.tile([1, CW], f32, tag="den", name="den")
                        nc.vector.tensor_copy(den[:], zps[i][D:D + 1, :])
                        rec = norm.tile([1, CW], f32, tag="rec", name="rec")
                        nc.vector.reciprocal_approx_fast(rec[:], den[:])
                        recb = norm.tile([D, CW], f32, tag="recb",
                                         name="recb")
                        nc.gpsimd.partition_broadcast(recb[:], rec[:])
                        lo = (h % 2) * D
                        nc.vector.tensor_mul(
                            zTm[lo:lo + D, h // 2, c * CW:(c + 1) * CW],
                            zps[i][:D], recb[:],
                        )
                while state["fi"] < len(fillers):
                    fillers[state["fi"]]()
                    state["fi"] += 1

            def outproj_task(c, tt, n):
                    t = 4 * c + tt
                    osb = work.tile([P, CW], f32, tag="osb", name="osb")
                    pp = ps_m.tile([P, CW], f32, tag="mps", name="ops")
                    for cc in range(2):
                        nc.tensor.matmul(
                            pp[:],
                            zTm[:, cc, t * P:(t + 1) * P],
                            wp_r[:, cc, n * CW:(n + 1) * CW],
                            start=(cc == 0), stop=(cc == 1),
                        )
                    nc.vector.tensor_copy(osb[:], pp[:])
                    nc.sync.dma_start(
                        out[t * P:(t + 1) * P, n * CW:(n + 1) * CW], osb[:]
                    )

            def prep_tasks(c):
                tasks = [
                    (lambda tt=tt: transpose_task(c, tt)) for tt in range(4)
                ]
                tasks += [(lambda oc=oc: qkproj_task(c, oc)) for oc in range(4)]
                tasks += [(lambda tt=tt: vproj_task(c, tt)) for tt in range(4)]
                return tasks

            # chunk 0 prep up front, then software-pipeline: during
            # attention(c), weave in outproj(c-1) and all prep for c+1.
            p0 = prep_tasks(0)
            wq_tasks = [(lambda fc=fc: load_wqk_task(fc)) for fc in range(FC)]
            wv_tasks = [(lambda fc=fc: load_wv_task(fc)) for fc in range(FC)]
            # transposes(0) first, weights woven behind them, then projs
            for task in p0[:4]:
                task()
            for task in wq_tasks + wv_tasks:
                task()
            for task in p0[4:]:
                task()
            # outproj(c) is shifted as late as possible so the long final
            # attention chunks (most insertion points) have filler work:
            # att0: wp+prep1, att1: prep2, att2: prep3+out0, att3: out1+out2
            out_t = [
                [(lambda tt=tt, cp=cp, n=n: outproj_task(cp, tt, n))
                 for tt in range(4) for n in range(2)]
                for cp in range(SQC)
            ]
            for c in range(SQC):
                fillers = []
                if c == 1:
                    fillers += [(lambda cc=cc: load_wp_task(cc))
                                for cc in range(2)]
                if c + 1 < SQC:
                    fillers += prep_tasks(c + 1)
                if c == 2:
                    fillers += out_t[0]
                if c == 3:
                    fillers += out_t[1] + out_t[2]
                attention(c, fillers)
            for task in out_t[3]:
                task()
    nc.compile()
    return nc


def make_in_maps(x, w_attn, b_attn, w_proj):
    x = np.ascontiguousarray(np.asarray(x, dtype=np.float32))
    w_attn = np.asarray(w_attn, dtype=np.float32)
    b_attn = np.asarray(b_attn, dtype=np.float32)
    w_proj = np.ascontiguousarray(np.asarray(w_proj, dtype=np.float32))
    scale = np.float32(1.0 / np.sqrt(D))
    in_maps = []
    for core in range(NCORES):
        b, g = divmod(core, 4)
        sl = slice(g * GD, (g + 1) * GD)
        wq = w_attn[:, sl] * scale
        wk = w_attn[:, F + g * GD:F + (g + 1) * GD]
        wqkm = np.ascontiguousarray(
            np.concatenate([wq, wk], axis=1), dtype=np.float32
        )
        wvm = np.ascontiguousarray(
            w_attn[:, 2 * F + g * GD:2 * F + (g + 1) * GD]
        )
        wpg = np.ascontiguousarray(w_proj[sl, :])
        bq = b_attn[sl] * scale
        bk = b_attn[F + g * GD:F + (g + 1) * GD]
        bqkm = np.ascontiguousarray(
            np.concatenate([bq, bk]).reshape(4, P).T, dtype=np.float32
        )
        in_maps.append(
            {"x": np.ascontiguousarray(x[b]), "wqk": wqkm, "wv": wvm,
             "wp": wpg, "bqk": bqkm}
        )
    return in_maps


def assemble(results, b_attn, b_proj, w_proj):
    b_attn = np.asarray(b_attn, dtype=np.float64)
    b_proj = np.asarray(b_proj, dtype=np.float64)
    w_proj = np.asarray(w_proj, dtype=np.float64)
    const = b_attn[2 * F:] @ w_proj + b_proj  # token-independent v-bias term
    full = np.empty((B, S, F), dtype=np.float32)
    for b in range(B):
        acc = results[4 * b]["out"].astype(np.float64)
        for g in range(1, 4):
            acc = acc + results[4 * b + g]["out"]
        full[b] = (acc + const).astype(np.float32)
    return full


def kernel(x, w_attn, b_attn, w_proj, b_proj):
    global _cached_nc
    if _cached_nc is None:
        _cached_nc = build_nc()
    in_maps = make_in_maps(x, w_attn, b_attn, w_proj)
    res = run_bass_kernel_spmd(
        _cached_nc, in_maps, core_ids=list(range(NCORES))
    )
    return assemble(res.results, b_attn, b_proj, w_proj)


# revision 29
# speedup vs baseline: 1.1214x; 1.0463x over previous
"""Multi-head causal attention block (B=2, S=2048, F=1024, H=16, D=64)
on 8 TRN2 NeuronCores.

Sharding: core = 4*b + g  (b = batch 0..1, g = head-group 0..3, 4 heads each).
Each core computes, for its batch and its 4 heads:
  qkv projection (columns of w_attn for its heads), causal attention,
  and the partial output projection (rows of w_proj for its heads).
Host sums the 4 per-group partials per batch and adds the bias constant
(b_proj + b_attn_v @ w_proj, which is token-independent).

On-chip dataflow ("orientation B" — scores transposed, no P-transposes of
the attention weights):
  xT   [f, s]   via PE transposes (4 per PSUM bank, batched copy-out)
  qkT  [dim, s] = wqk^T @ xT; chunks [q_h0|q_h1],[q_h2|q_h3],[k_h0|k_h1],[k_h2|k_h3]
  v    [s, d]   direct orientation, +ones column per head (denominator row)
  attention per head h, sq-chunk c (512 wide), sk tile t<=diag:
    sT = matmul(lhsT=kT_h[:,t], rhs=qT_h[:,chunk])  [sk=128, sq<=512] PSUM
    (pairs of t share one 2-bank PSUM tile; one exp op per pair)
    exp on ACT -> SBUF f32r; causal triangle zeroed by GPSIMD affine_select
    zT'[65, 512] += v_ones_h[:,t].T @ expP  (PSUM accumulate; row 64 = denom)
    normalize: z = zT'[:64] * bcast(approx_recip(zT'[64]))
  out partial [s, f] = zTm.T @ wp

Diag tiles compute only the valid sq range (width 512-off), so there is no
wasted score/exp/AV work beyond the masked 128x128 triangle.

Everything is emitted chunk-pipelined (transpose(c) -> proj(c) ->
attention(c) -> outproj(c)) so the Tile scheduler can overlap phases and
keep the PE dense (HAM stays un-throttled).

All matmuls run in float32r (full-rate fp32; ~2^-14 operand rounding).
"""

import numpy as np

import concourse.mybir as mybir
import concourse.tile as tile
from concourse import bacc
from concourse.bass_utils import run_bass_kernel_spmd
from concourse.masks import make_identity

B, S, F, H, D = 2, 2048, 1024, 16, 64
P = 128
NCORES = 8
HPC = 4  # heads per core
GD = HPC * D  # 256 dims per head group
ST = S // P  # 16 sequence tiles
FC = F // P  # 8 feature chunks
SQC = 4  # sq chunks of 512
CW = 512  # chunk width
NEG = -1.0e9

f32 = mybir.dt.float32
f32r = mybir.dt.float32r

_cached_nc = None


def build_nc():
    nc = bacc.Bacc("TRN2", target_bir_lowering=False, debug=False,
                   num_devices=NCORES)
    x = nc.dram_tensor("x", [S, F], f32, kind="ExternalInput")
    wqk = nc.dram_tensor("wqk", [F, 2 * GD], f32, kind="ExternalInput")
    wv = nc.dram_tensor("wv", [F, GD], f32, kind="ExternalInput")
    wp = nc.dram_tensor("wp", [GD, F], f32, kind="ExternalInput")
    bqk = nc.dram_tensor("bqk", [P, 4], f32, kind="ExternalInput")
    out = nc.dram_tensor("out", [S, F], f32, kind="ExternalOutput")

    with tile.TileContext(nc) as tc:
        with (
            tc.tile_pool(name="consts", bufs=1) as consts,
            tc.tile_pool(name="stage", bufs=1) as stage,
            tc.tile_pool(name="work", bufs=2) as work,
            tc.tile_pool(name="xtp", bufs=3) as xtp,
            tc.tile_pool(name="eps", bufs=4) as eps,
            tc.tile_pool(name="norm", bufs=1) as norm,
            tc.tile_pool(name="ps_s", bufs=2, space="PSUM") as ps_s,
            tc.tile_pool(name="ps_z", bufs=2, space="PSUM") as ps_z,
            tc.tile_pool(name="ps_m", bufs=2, space="PSUM") as ps_m,
        ):
            # ---- constants ----
            ident = consts.tile([P, P], f32)
            make_identity(nc, ident[:])
            ones = consts.tile([P, 1], f32)
            nc.vector.memset(ones[:], 1.0)
            bqk_sb = consts.tile([P, 4], f32)
            nc.sync.dma_start(bqk_sb[:], bqk[:])
            # additive causal triangle: keep iff jloc >= i, else -1e9
            mask128 = consts.tile([P, P], f32)
            nc.gpsimd.memset(mask128[:], 0.0)
            nc.gpsimd.affine_select(
                out=mask128[:], in_=mask128[:],
                compare_op=mybir.AluOpType.is_ge,
                fill=NEG, base=0,
                pattern=[[1, P]], channel_multiplier=-1,
            )

            # ---- round weights to f32r (streamed through small tiles) ----
            wqk_r = stage.tile([P, FC, 2 * GD], f32r, tag="wqk_r", name="wqk_r")
            wv_r = stage.tile([P, FC, GD], f32r, tag="wv_r", name="wv_r")
            wp_r = stage.tile([P, 2, F], f32r, tag="wp_r", name="wp_r")
            def load_wqk_task(fc):
                wt = work.tile([P, 2 * GD], f32, tag="wtmp", name="wt_qk")
                nc.sync.dma_start(wt[:], wqk[fc * P:(fc + 1) * P, :])
                nc.vector.tensor_copy(wqk_r[:, fc, :], wt[:])

            def load_wv_task(fc):
                wt = work.tile([P, 2 * GD], f32, tag="wtmp", name="wt_v")
                nc.sync.dma_start(wt[:, :GD], wv[fc * P:(fc + 1) * P, :])
                nc.vector.tensor_copy(wv_r[:, fc, :], wt[:, :GD])

            def load_wp_task(cc):
                for hh in range(2):
                    wt = work.tile([P, 2 * GD], f32, tag="wtmp", name="wt_p")
                    nc.sync.dma_start(
                        wt[:], wp[cc * P:(cc + 1) * P,
                                  hh * CW:(hh + 1) * CW])
                    nc.vector.tensor_copy(
                        wp_r[:, cc, hh * CW:(hh + 1) * CW], wt[:])

            # ---- persistent activations ----
            xT = stage.tile([P, FC, S], f32r, tag="xT", name="xT")
            qkT = stage.tile([P, 4, S], f32r, tag="qkT", name="qkT")
            vt = stage.tile([P, HPC, ST, D + 1], f32r, tag="vt", name="vt")
            zTm = stage.tile([P, 2, S], f32r, tag="zTm", name="zTm")
            for h in range(HPC):
                nc.vector.tensor_copy(
                    vt[:, h, :, D:D + 1],
                    ones[:, None, :].to_broadcast((P, ST, 1)),
                )

            def transpose_task(c, tt):
                t = 4 * c + tt
                xt_ = xtp.tile([P, F], f32, tag="xtile", name="xtile")
                hw = F // 2
                for half in range(2):
                    nc.sync.dma_start(
                        xt_[:, half * hw:(half + 1) * hw],
                        x[t * P:(t + 1) * P, half * hw:(half + 1) * hw],
                    )
                for half in range(2):
                    pp = ps_m.tile([P, CW], f32, tag="mps", name="tps")
                    for q in range(4):
                        fc = half * 4 + q
                        nc.tensor.transpose(
                            pp[:, q * P:(q + 1) * P],
                            xt_[:, fc * P:(fc + 1) * P],
                            ident[:],
                        )
                    nc.vector.tensor_copy(
                        xT[:, half * 4:half * 4 + 4,
                           t * P:(t + 1) * P],
                        pp[:].rearrange("p (f q) -> p f q", f=4),
                    )

            def qkproj_task(c, oc):
                    pp = ps_m.tile([P, CW], f32, tag="mps", name="qkps")
                    for fc in range(FC):
                        nc.tensor.matmul(
                            pp[:],
                            wqk_r[:, fc, oc * P:(oc + 1) * P],
                            xT[:, fc, c * CW:(c + 1) * CW],
                            start=(fc == 0), stop=(fc == FC - 1),
                        )
                    nc.vector.tensor_tensor(
                        qkT[:, oc, c * CW:(c + 1) * CW], pp[:],
                        bqk_sb[:, oc:oc + 1].to_broadcast((P, CW)),
                        mybir.AluOpType.add,
                    )

            def vproj_task(c, tt):
                    t = 4 * c + tt
                    pp = ps_m.tile([P, GD], f32, tag="mps", name="vps")
                    for fc in range(FC):
                        nc.tensor.matmul(
                            pp[:],
                            xT[:, fc, t * P:(t + 1) * P],
                            wv_r[:, fc, :],
                            start=(fc == 0), stop=(fc == FC - 1),
                        )
                    nc.vector.tensor_copy(
                        vt[:, :, t, :D],
                        pp[:].rearrange("p (h d) -> p h d", h=HPC),
                    )

            def av(zp, h, t, ep_ap, col0, ncols, start, stop):
                nc.tensor.matmul(
                    zp[:D + 1, col0:col0 + ncols],
                    vt[:, h, t, :],
                    ep_ap,
                    start=start, stop=stop,
                    skip_group_check=True,
                )

            def scores(sp_ap, h, t, c, q0, qw):
                lo = (h % 2) * D
                nc.tensor.matmul(
                    sp_ap,
                    qkT[lo:lo + D, 2 + h // 2, t * P:(t + 1) * P],
                    qkT[lo:lo + D, h // 2, c * CW + q0:c * CW + q0 + qw],
                    start=True, stop=True,
                    skip_group_check=True,
                )

            def diag_mask(sp_ap):
                nc.vector.tensor_add(sp_ap, sp_ap, mask128[:])

            def attention(c, fillers):
                # insertion points: one after each head's exp emission
                npts = 2 * (2 * c + 2) * 2
                state = {"fi": 0, "pt": 0}

                def fill():
                    state["pt"] += 1
                    left = npts - state["pt"] + 1
                    remaining = len(fillers) - state["fi"]
                    k = (remaining + left - 1) // left if left > 0 else remaining
                    for _ in range(k):
                        fillers[state["fi"]]()
                        state["fi"] += 1

                for hp in range(2):
                    heads = (2 * hp, 2 * hp + 1)
                    zps = [
                        ps_z.tile([P, CW], f32, tag="zps", name=f"zps{i}")
                        for i in range(2)
                    ]
                    # off-diagonal pairs (full width); both heads' score
                    # matmuls issued adjacently so the K=64 matmuls pack
                    # into disjoint PE row groups and run concurrently.
                    for pair in range(2 * c):
                        t0, t1 = 2 * pair, 2 * pair + 1
                        sp2 = [
                            ps_s.tile([P, 2 * CW], f32, tag="sps",
                                      name=f"sps{i}")
                            for i in range(2)
                        ]
                        for i, h in enumerate(heads):
                            scores(sp2[i][:, 0:CW], h, t0, c, 0, CW)
                            scores(sp2[i][:, CW:2 * CW], h, t1, c, 0, CW)
                        ep2 = []
                        for i, h in enumerate(heads):
                            ep = eps.tile([P, 2 * CW], f32r, tag="ep",
                                          name=f"ep{i}")
                            nc.scalar.activation(
                                ep[:], sp2[i][:],
                                mybir.ActivationFunctionType.Exp,
                            )
                            ep2.append(ep)
                        fill()
                        first = (t0 == 0)
                        for i, h in enumerate(heads):
                            av(zps[i], h, t0, ep2[i][:, 0:CW], 0, CW,
                               first, False)
                            av(zps[i], h, t1, ep2[i][:, CW:2 * CW], 0, CW,
                               False, False)
                        fill()
                    # diagonal pairs: widths (512, 384) and (256, 128)
                    for dp in range(2):
                        ta, tb = 4 * c + 2 * dp, 4 * c + 2 * dp + 1
                        offa, offb = 2 * dp * P, (2 * dp + 1) * P
                        wa, wb = CW - offa, CW - offb
                        sp2 = [
                            ps_s.tile([P, 2 * CW], f32, tag="sps",
                                      name=f"sps{i}")
                            for i in range(2)
                        ]
                        for i, h in enumerate(heads):
                            scores(sp2[i][:, 0:wa], h, ta, c, offa, wa)
                            scores(sp2[i][:, wa:wa + wb], h, tb, c, offb, wb)
                        ep2 = []
                        for i, h in enumerate(heads):
                            diag_mask(sp2[i][:, 0:P])
                            diag_mask(sp2[i][:, wa:wa + P])
                            ep = eps.tile([P, 2 * CW], f32r, tag="ep",
                                          name=f"ep{i}")
                            nc.scalar.activation(
                                ep[:, 0:wa + wb], sp2[i][:, 0:wa + wb],
                                mybir.ActivationFunctionType.Exp,
                            )
                            ep2.append(ep)
                        fill()
                        first = (c == 0 and dp == 0)
                        for i, h in enumerate(heads):
                            av(zps[i], h, ta, ep2[i][:, 0:wa], offa, wa,
                               first, False)
                            av(zps[i], h, tb, ep2[i][:, wa:wa + wb], offb,
                               wb, False, (dp == 1))
                        fill()
                    # normalize
                    for i, h in enumerate(heads):
                        den = norm.tile([1, CW], f32, tag="den", name="den")
                        nc.vector.tensor_copy(den[:], zps[i][D:D + 1, :])
                        rec = norm.tile([1, CW], f32, tag="rec", name="rec")
                        nc.vector.reciprocal_approx_fast(rec[:], den[:])
                        recb = norm.tile([D, CW], f32, tag="recb",
                                         name="recb")
                        nc.gpsimd.partition_broadcast(recb[:], rec[:])
                        lo = (h % 2) * D
                        nc.vector.tensor_mul(
                            zTm[lo:lo + D, h // 2, c * CW:(c + 1) * CW],
                            zps[i][:D], recb[:],
                        )
                while state["fi"] < len(fillers):
                    fillers[state["fi"]]()
                    state["fi"] += 1

            def outproj_task(c, tt, n):
                    t = 4 * c + tt
                    osb = work.tile([P, CW], f32, tag="osb", name="osb")
                    pp = ps_m.tile([P, CW], f32, tag="mps", name="ops")
                    for cc in range(2):
                        nc.tensor.matmul(
                            pp[:],
                            zTm[:, cc, t * P:(t + 1) * P],
                            wp_r[:, cc, n * CW:(n + 1) * CW],
                            start=(cc == 0), stop=(cc == 1),
                        )
                    nc.vector.tensor_copy(osb[:], pp[:])
                    nc.sync.dma_start(
                        out[t * P:(t + 1) * P, n * CW:(n + 1) * CW], osb[:]
                    )

            def prep_tasks(c):
                tasks = [
                    (lambda tt=tt: transpose_task(c, tt)) for tt in range(4)
                ]
                tasks += [(lambda oc=oc: qkproj_task(c, oc)) for oc in range(4)]
                tasks += [(lambda tt=tt: vproj_task(c, tt)) for tt in range(4)]
                return tasks

            # chunk 0 prep up front, then software-pipeline: during
            # attention(c), weave in outproj(c-1) and all prep for c+1.
            p0 = prep_tasks(0)
            wq_tasks = [(lambda fc=fc: load_wqk_task(fc)) for fc in range(FC)]
            wv_tasks = [(lambda fc=fc: load_wv_task(fc)) for fc in range(FC)]
            # transposes(0) first, weights woven behind them, then projs
            for task in p0[:4]:
                task()
            for task in wq_tasks + wv_tasks:
                task()
            for task in p0[4:]:
                task()
            # outproj(c) is shifted as late as possible so the long final
            # attention chunks (most insertion points) have filler work:
            # att0: wp+prep1, att1: prep2, att2: prep3+out0, att3: out1+out2
            out_t = [
                [(lambda tt=tt, cp=cp, n=n: outproj_task(cp, tt, n))
                 for tt in range(4) for n in range(2)]
                for cp in range(SQC)
            ]
            for c in range(SQC):
                fillers = []
                if c == 1:
                    fillers += [(lambda cc=cc: load_wp_task(cc))
                                for cc in range(2)]
                if c + 1 < SQC:
                    fillers += prep_tasks(c + 1)
                if c == 2:
                    fillers += out_t[0]
                if c == 3:
                    fillers += out_t[1] + out_t[2]
                attention(c, fillers)
            for task in out_t[3]:
                task()
    nc.compile()
    return nc


def make_in_maps(x, w_attn, b_attn, w_proj):
    x = np.ascontiguousarray(np.asarray(x, dtype=np.float32))
    w_attn = np.asarray(w_attn, dtype=np.float32)
    b_attn = np.asarray(b_attn, dtype=np.float32)
    w_proj = np.ascontiguousarray(np.asarray(w_proj, dtype=np.float32))
    scale = np.float32(1.0 / np.sqrt(D))
    in_maps = []
    for core in range(NCORES):
        b, g = divmod(core, 4)
        sl = slice(g * GD, (g + 1) * GD)
        wq = w_attn[:, sl] * scale
        wk = w_attn[:, F + g * GD:F + (g + 1) * GD]
        wqkm = np.ascontiguousarray(
            np.concatenate([wq, wk], axis=1), dtype=np.float32
        )
        wvm = np.ascontiguousarray(
            w_attn[:, 2 * F + g * GD:2 * F + (g + 1) * GD]
        )
        wpg = np.ascontiguousarray(w_proj[sl, :])
        bq = b_attn[sl] * scale
        bk = b_attn[F + g * GD:F + (g + 1) * GD]
        bqkm = np.ascontiguousarray(
            np.concatenate([bq, bk]).reshape(4, P).T, dtype=np.float32
        )
        in_maps.append(
            {"x": np.ascontiguousarray(x[b]), "wqk": wqkm, "wv": wvm,
             "wp": wpg, "bqk": bqkm}
        )
    return in_maps


def assemble(results, b_attn, b_proj, w_proj):
    b_attn = np.asarray(b_attn, dtype=np.float64)
    b_proj = np.asarray(b_proj, dtype=np.float64)
    w_proj = np.asarray(w_proj, dtype=np.float64)
    const = b_attn[2 * F:] @ w_proj + b_proj  # token-independent v-bias term
    full = np.empty((B, S, F), dtype=np.float32)
    for b in range(B):
        acc = results[4 * b]["out"].astype(np.float64)
        for g in range(1, 4):
            acc = acc + results[4 * b + g]["out"]
        full[b] = (acc + const).astype(np.float32)
    return full


def kernel(x, w_attn, b_attn, w_proj, b_proj):
    global _cached_nc
    if _cached_nc is None:
        _cached_nc = build_nc()
    in_maps = make_in_maps(x, w_attn, b_attn, w_proj)
    res = run_bass_kernel_spmd(
        _cached_nc, in_maps, core_ids=list(range(NCORES))
    )
    return assemble(res.results, b_attn, b_proj, w_proj)
